# revision 9
# baseline (speedup 1.0000x reference)
"""Trainium2 Bass kernel for nn_FCClassifier (predictive-coding FC network).

Data-parallel over batch (1024 -> 128 rows/core on 8 cores); state in SBUF as
[128, width] fp32. Per settling step:
  top-down:  pred_{li-1} = tanh(x_li) @ W_li^T   (bf16 operands, fp32 PSUM acc)
             e_{li-1} = x_{li-1} - pred + noise_eff
  bottom-up: g_li = e_{li-1} @ W_li ; x_li += 0.1*(g*(1-tanh(x_li)^2) - e_li)
noise_eff is host-precomputed (exact jax threefry bits, 0.034*temp scaling and
bias b_li folded in). Weights stream from HBM every step as two pre-swizzled
bf16 blobs (W^T layout for top-down, natural for bottom-up/init). Transposed
activations (matmul stationary operands) are made on-chip with PE transposes.
Steps run in a hardware For_i loop; only the noise DMA offset is step-dependent.
"""
import contextlib
import numpy as np
import ml_dtypes

import concourse.bass as bass
import concourse.tile as tile
from concourse import bacc, mybir
from concourse import bass_utils

SIZES = [3072, 4096, 4096, 2048, 1000]
BATCH = 1024
GAMMA = 0.1
NOISE_SCALE = 0.034
N_CORES = 8
ROWS = BATCH // N_CORES  # 128

BF16 = mybir.dt.bfloat16
F32 = mybir.dt.float32
NPBF16 = ml_dtypes.bfloat16

NGROUP = 4   # accumulator banks per matmul group
CHUNK = 512  # moving free dim / vector chunk
KSEG = 2     # k-tiles per weight-stream DMA segment

W_TOT = sum(SIZES)        # 14312
E_TOT = sum(SIZES[:4])    # 13312
XOFS = np.cumsum([0] + SIZES)
EOFS = np.cumsum([0] + SIZES[:4])


def _cdiv(a, b):
    return (a + b - 1) // b


def _chunks(total, size):
    return [(o, min(size, total - o)) for o in range(0, total, size)]


def _groups(lst, n):
    return [lst[i:i + n] for i in range(0, len(lst), n)]


def _sweep_dims(kind, li):
    if kind == "fwd":
        return SIZES[li], SIZES[li - 1]   # K, N
    return SIZES[li - 1], SIZES[li]


N_BLOCKS = sum(_cdiv(_sweep_dims("fwd", li)[0], 128) * len(_chunks(_sweep_dims("fwd", li)[1], CHUNK))
               for li in (1, 2, 3, 4))  # same count for both sweeps (608)


# ---------------------------------------------------------------- host prep

def _noise_eff(steps, bs):
    """[steps*1024, 13312] bf16: exact reference noise * scale - bias folds."""
    import jax, jax.numpy as jnp
    cpu = jax.devices("cpu")[0]
    with jax.default_device(cpu):
        nkey = jax.random.key(42)
        rows = []
        for i in range(steps):
            temp = np.float32(1.0 - np.float32(i) / steps)
            pieces = []
            for lo in range(4):
                k = jax.random.fold_in(jax.random.fold_in(nkey, i), lo)
                nz = np.asarray(jax.random.normal(k, (BATCH, SIZES[lo]), jnp.float32))
                nz = nz * np.float32(NOISE_SCALE) * temp - bs[lo][None, :]
                pieces.append(nz)
            rows.append(np.concatenate(pieces, axis=1))
        return np.stack(rows).astype(NPBF16)  # [steps, 1024, E_TOT]


def _pack_blob(Ws, kind):
    """Pre-swizzled weight blob in exact consumption order."""
    blocks = []
    order = (4, 3, 2, 1) if kind == "fwd" else (1, 2, 3, 4)
    for li in order:
        Wm = Ws[li - 1].T if kind == "fwd" else Ws[li - 1]
        K, N = Wm.shape
        kt_n = _cdiv(K, 128)
        for grp in _groups(_chunks(N, CHUNK), NGROUP):
            for seg0 in range(0, kt_n, KSEG):
                for kt in range(seg0, min(seg0 + KSEG, kt_n)):
                    k0, kw = kt * 128, min(128, K - kt * 128)
                    for (n0, nw) in grp:
                        blk = np.zeros((128, CHUNK), np.float32)
                        blk[:kw, :nw] = Wm[k0:k0 + kw, n0:n0 + nw]
                        blocks.append(blk.reshape(1, -1))
    return np.concatenate(blocks, 0).astype(NPBF16)


# ---------------------------------------------------------------- builder

class _C:
    pass


def _emit_layer_mm(c, li, kind, lhsT, blob, ofs, out_cb):
    """Matmuls for one layer of a sweep, streaming weights in KSEG segments."""
    nc = c.nc
    K, N = _sweep_dims(kind, li)
    kt_n = _cdiv(K, 128)
    for grp in _groups(_chunks(N, CHUNK), NGROUP):
        accs = [c.apool.tile([128, CHUNK], F32, tag="acc", name="acc") for _ in grp]
        for seg0 in range(0, kt_n, KSEG):
            seg_n = min(KSEG, kt_n - seg0)
            nblk = seg_n * len(grp)
            wt = c.wpool.tile([128, KSEG * NGROUP * CHUNK], BF16, tag="wstream")
            nc.sync.dma_start(
                wt[:].rearrange("p (b f) -> p b f", f=CHUNK)[:, :nblk],
                blob[ofs[0]:ofs[0] + nblk].rearrange("b (p f) -> p b f", p=128))
            ofs[0] += nblk
            for si in range(seg_n):
                kt = seg0 + si
                kw = min(128, K - kt * 128)
                for gi, (n0, nw) in enumerate(grp):
                    bi = si * len(grp) + gi
                    nc.tensor.matmul(
                        accs[gi][:, :nw],
                        lhsT[:kw, kt * 128:kt * 128 + 128],
                        wt[:kw, bi * CHUNK:bi * CHUNK + nw],
                        start=(kt == 0), stop=(kt == kt_n - 1))
        for gi, (n0, nw) in enumerate(grp):
            out_cb(li, n0, nw, accs[gi])


def _emit_transpose(c, dst, src, width):
    """PE-transpose [128, width] bf16 src -> dst [128, kt_n*128] k-major tiles."""
    nc = c.nc
    kt_n = _cdiv(width, 128)
    for base in range(0, kt_n, 8):
        nt = min(8, kt_n - base)
        pt = c.tpool.tile([128, 8 * 128], BF16, tag="tr")
        kws = []
        for j in range(nt):
            kt = base + j
            kw = min(128, width - kt * 128)
            kws.append(kw)
            nc.tensor.transpose(
                pt[:kw, j * 128:j * 128 + 128],
                src[:, kt * 128:kt * 128 + kw],
                c.ident[:, :])
        if all(k == 128 for k in kws):
            nc.vector.tensor_copy(dst[:, base * 128:(base + nt) * 128],
                                  pt[:, :nt * 128])
        else:
            nfull = sum(1 for k in kws if k == 128)
            if nfull:
                nc.vector.tensor_copy(dst[:, base * 128:(base + nfull) * 128],
                                      pt[:, :nfull * 128])
            for j in range(nfull, nt):
                kw = kws[j]
                nc.vector.tensor_copy(
                    dst[:kw, (base + j) * 128:(base + j) * 128 + 128],
                    pt[:kw, j * 128:j * 128 + 128])


def build(steps):
    nc = bacc.Bacc("TRN2", target_bir_lowering=False, debug=False,
                   num_devices=N_CORES)
    c = _C()
    c.nc = nc

    obs_d = nc.dram_tensor("obs", [ROWS, SIZES[0]], F32, kind="ExternalInput").ap()
    obsT_d = nc.dram_tensor("obsT", [SIZES[0] // 128, 128 * ROWS], BF16,
                            kind="ExternalInput").ap()
    wt_d = nc.dram_tensor("wt_blob", [N_BLOCKS, 128 * CHUNK], BF16,
                          kind="ExternalInput").ap()
    wn_d = nc.dram_tensor("wn_blob", [N_BLOCKS, 128 * CHUNK], BF16,
                          kind="ExternalInput").ap()
    ident_d = nc.dram_tensor("ident", [128, 128], BF16, kind="ExternalInput").ap()
    noise_d = nc.dram_tensor("noise", [steps * ROWS, E_TOT], BF16,
                             kind="ExternalInput").ap()
    out_d = nc.dram_tensor("out", [ROWS, SIZES[4]], F32, kind="ExternalOutput").ap()

    with tile.TileContext(nc) as tc, contextlib.ExitStack() as st:
        c.wpool = st.enter_context(tc.tile_pool(name="wstream", bufs=4))
        c.apool = st.enter_context(tc.tile_pool(name="acc", bufs=6, space="PSUM"))
        c.tpool = st.enter_context(tc.tile_pool(name="tr", bufs=2, space="PSUM"))
        sp = st.enter_context(tc.tile_pool(name="state", bufs=1))
        sc = st.enter_context(tc.tile_pool(name="scratch", bufs=2))
        tp = st.enter_context(tc.tile_pool(name="tTpool", bufs=2))

        x = sp.tile([128, W_TOT], F32)
        e = sp.tile([128, E_TOT], BF16)
        ident = sp.tile([128, 128], BF16)
        c.ident = ident
        nc.sync.dma_start(ident[:], ident_d)
        eT = {lo: sp.tile([128, _cdiv(SIZES[lo], 128) * 128], BF16, tag=f"eT{lo}", name=f"eT{lo}")
              for lo in range(4)}

        def x_ap(li, n0=0, nw=None):
            nw = SIZES[li] if nw is None else nw
            o = int(XOFS[li]) + n0
            return x[:, o:o + nw]

        def e_ap(lo, n0=0, nw=None):
            nw = SIZES[lo] if nw is None else nw
            o = int(EOFS[lo]) + n0
            return e[:, o:o + nw]

        nc.sync.dma_start(x_ap(0), obs_d)

        # ---------------- init: x_li = x_{li-1} @ W_li
        def init_out(li, n0, nw, acc):
            nc.vector.tensor_copy(x_ap(li, n0, nw), acc[:, :nw])

        init_ofs = [0]
        xT_prev = sp.tile([128, 32 * 128], BF16, tag="xTinit", name="xTinit")
        nc.sync.dma_start(
            xT_prev[:, :SIZES[0]].rearrange("p (k f) -> p k f", f=128),
            obsT_d.rearrange("k (p f) -> p k f", p=128))
        for li in (1, 2, 3, 4):
            _emit_layer_mm(c, li, "init", xT_prev, wn_d, init_ofs, init_out)
            if li < 4:
                xT_prev = sp.tile([128, 32 * 128], BF16, tag="xTinit", name="xTinit")
                for (s0, swd) in _chunks(SIZES[li], 1024):
                    xb = sc.tile([128, 1024], BF16, tag="xbinit", name="xbinit")
                    nc.vector.tensor_copy(xb[:, :swd], x_ap(li, s0, swd))
                    _emit_transpose(c, xT_prev[:, s0:s0 + _cdiv(swd, 128) * 128], xb[:, :swd], swd)

        # ---------------- settling steps
        def step_body(i):
            fwd_ofs = [0]
            for li in (4, 3, 2, 1):
                tT = tp.tile([128, 32 * 128], BF16, tag="tT", name="tT")
                for (s0, swd) in _chunks(SIZES[li], 1024):
                    tb = sc.tile([128, 1024], BF16, tag="tcast", name="tcast")
                    for (n0, nw) in _chunks(swd, CHUNK):
                        nc.scalar.activation(tb[:, n0:n0 + nw],
                                             x_ap(li, s0 + n0, nw),
                                             mybir.ActivationFunctionType.Tanh)
                    _emit_transpose(c, tT[:, s0:s0 + _cdiv(swd, 128) * 128], tb[:, :swd], swd)

                def fwd_out(li_, n0, nw, acc, _lo=li - 1):
                    nz = sc.tile([128, CHUNK], BF16, tag="nz")
                    nc.sync.dma_start(
                        nz[:, :nw],
                        noise_d[bass.ts(i, ROWS),
                                int(EOFS[_lo]) + n0:int(EOFS[_lo]) + n0 + nw])
                    # e = (pred * -1 + x) + noise_eff
                    nc.vector.scalar_tensor_tensor(
                        e_ap(_lo, n0, nw), acc[:, :nw], -1.0, x_ap(_lo, n0, nw),
                        mybir.AluOpType.mult, mybir.AluOpType.add)
                    nc.vector.tensor_add(e_ap(_lo, n0, nw), e_ap(_lo, n0, nw),
                                         nz[:, :nw])

                _emit_layer_mm(c, li, "fwd", tT, wt_d, fwd_ofs, fwd_out)
                _emit_transpose(c, eT[li - 1], e_ap(li - 1), SIZES[li - 1])

            bwd_ofs = [0]

            def bwd_out(li, n0, nw, acc):
                t2 = sc.tile([128, CHUNK], F32, tag="t2")
                nc.scalar.activation(t2[:, :nw], x_ap(li, n0, nw),
                                     mybir.ActivationFunctionType.Tanh)
                nc.scalar.activation(t2[:, :nw], t2[:, :nw],
                                     mybir.ActivationFunctionType.Square)
                nc.vector.tensor_scalar(t2[:, :nw], t2[:, :nw], -1.0, 1.0,
                                        mybir.AluOpType.mult, mybir.AluOpType.add)
                gd = sc.tile([128, CHUNK], F32, tag="gd")
                nc.vector.tensor_mul(gd[:, :nw], acc[:, :nw], t2[:, :nw])
                if li < 4:  # e4 is identically zero in the reference
                    nc.vector.scalar_tensor_tensor(
                        gd[:, :nw], e_ap(li, n0, nw), -1.0, gd[:, :nw],
                        mybir.AluOpType.mult, mybir.AluOpType.add)
                nc.vector.scalar_tensor_tensor(
                    x_ap(li, n0, nw), gd[:, :nw], GAMMA, x_ap(li, n0, nw),
                    mybir.AluOpType.mult, mybir.AluOpType.add)

            for li in (1, 2, 3, 4):
                _emit_layer_mm(c, li, "bwd", eT[li - 1], wn_d, bwd_ofs, bwd_out)

        with tc.For_i(0, steps, 1, hint_engines=(mybir.EngineType.PE, mybir.EngineType.DVE, mybir.EngineType.Activation, mybir.EngineType.SP), staggered_reset=True) as i:
            step_body(i)

        nc.sync.dma_start(out_d, x_ap(4))
    nc.finalize()
    return nc


# ---------------------------------------------------------------- entry

_CACHE = {}


def kernel(**inputs):
    obs = np.asarray(inputs["obs"], np.float32)
    Ws = [np.asarray(inputs[f"W{i}"], np.float32) for i in range(1, 5)]
    bs = [np.asarray(inputs[f"b{i}"], np.float32) for i in range(1, 5)]
    steps = int(inputs["steps"])
    assert obs.shape == (BATCH, SIZES[0])

    if steps not in _CACHE:
        _CACHE[steps] = build(steps)
    nc = _CACHE[steps]

    noise = _noise_eff(steps, bs)  # [steps, 1024, E_TOT] bf16
    wt_blob = _pack_blob(Ws, "fwd")
    wn_blob = _pack_blob(Ws, "bwd")
    ident = np.eye(128, dtype=NPBF16)

    in_maps = []
    for cx in range(N_CORES):
        r0 = cx * ROWS
        obs_c = np.ascontiguousarray(obs[r0:r0 + ROWS])
        obsT_c = np.ascontiguousarray(
            obs_c.T.astype(NPBF16).reshape(SIZES[0] // 128, 128 * ROWS))
        nz_c = np.ascontiguousarray(
            noise[:, r0:r0 + ROWS, :]).reshape(steps * ROWS, E_TOT)
        in_maps.append({
            "obs": obs_c, "obsT": obsT_c, "wt_blob": wt_blob,
            "wn_blob": wn_blob, "ident": ident, "noise": nz_c,
        })

    res = bass_utils.run_bass_kernel_spmd(
        nc, in_maps, core_ids=list(range(N_CORES)), trace=False)
    return np.concatenate(
        [res.results[cx]["out"] for cx in range(N_CORES)], 0).astype(np.float32)



# revision 10
# speedup vs baseline: 1.0599x; 1.0599x over previous
"""Trainium2 Bass kernel for nn_FCClassifier — tensor-parallel over features.

Every layer's features are sharded 8-ways (all sizes divide exactly; layer 4's
1000 pads to 128/core). State lives TRANSPOSED in SBUF: x (fp32, scaled x16)
and e (fp8 e4m3, scaled x16) as [128-feature, 1024-batch] tiles — no on-chip
transposes anywhere. Weights are stationary matmul operands resident in SBUF
(fwd W^T slices + bwd W4; bwd W1-3 stream from HBM each step, ~9.4MB).

Cross-core exchange is 4 fused AllGathers per step (collectives are ~50us
fixed-latency here, so fewer+bigger wins, each hidden under a layer's matmuls):
  AG(t4), AG(t1..t3)   after the step's tanh pass (fp8)
  AG(e3,e2)            after fwd4+fwd3   -> feeds bwd4, bwd3
  AG(e1,e0)            after fwd2+fwd1   -> feeds bwd2, bwd1
Gathered buffers live in Shared DRAM and stream back as moving matmul operands.
Steps are python-unrolled (collectives inside tc.For_i crash this runtime).
The x16 scaling keeps e*16 inside e4m3 range with unchanged update arithmetic;
tanh un-scales via the activation input scale; the host divides output by 16.
"""
import contextlib
import numpy as np
import ml_dtypes

import concourse.bass as bass
import concourse.tile as tile
from concourse import bacc, mybir
from concourse import bass_utils

SIZES = [3072, 4096, 4096, 2048, 1000]
BATCH = 1024
GAMMA = 0.1
NOISE_SCALE = 0.034
N_CORES = 8
SL = [s // N_CORES for s in SIZES]            # real slice: 384,512,512,256,125
SLP = [384, 512, 512, 256, 128]               # padded slice (partition tiles)
XT = [s // 128 for s in SLP]                  # tiles per slice: 3,4,4,2,1
KT_F = [0] + [-(-s // 128) for s in SIZES[1:]]  # fwd k-tiles: -,32,32,16,8
KT_B = [0] + [SIZES[li - 1] // 128 for li in range(1, 5)]  # bwd: -,24,32,32,16

BF16 = mybir.dt.bfloat16
F32 = mybir.dt.float32
F8 = mybir.dt.float8e4
NPBF16 = ml_dtypes.bfloat16
NPF8 = ml_dtypes.float8_e4m3

XS = 16.0      # x/e state scale (keeps e*16 in e4m3 range)
WSEG = 4       # bwd-weight k-tiles per DMA
RG = [list(range(N_CORES))]

# streamed bwd weight blob regions (li 1..3), kt-major it-minor [128,128] blocks
WB_OFS = {}
_o = 0
for _li in (1, 2, 3):
    WB_OFS[_li] = _o
    _o += KT_B[_li] * XT[_li]
N_WBS = _o  # 96+128+64 = 288

# fused t-gather (t1,t2,t3): per-core rows and layer offsets
TOFS = {1: 0, 2: SL[1], 3: SL[1] + SL[2]}
TROWS = SL[1] + SL[2] + SL[3]          # 1280
# fused e-gathers: (e3,e2) and (e1,e0)
E32OFS = {3: 0, 2: SL[3]}
E32ROWS = SL[3] + SL[2]                # 768
E10OFS = {1: 0, 0: SL[1]}
E10ROWS = SL[1] + SL[0]                # 896

NOFS = np.cumsum([0] + SL[:4])  # e-slice row offsets inside a step's noise blk
NROWS = int(NOFS[4])            # 1664


def build(steps):
    nc = bacc.Bacc("TRN2", target_bir_lowering=False, debug=False,
                   num_devices=N_CORES)

    obsT_d = nc.dram_tensor("obsT", [SLP[0], BATCH], F32,
                            kind="ExternalInput").ap()
    wf_d = {li: nc.dram_tensor(f"wf{li}", [KT_F[li] * 128, SLP[li - 1]], BF16,
                               kind="ExternalInput").ap() for li in (1, 2, 3, 4)}
    wb4_d = nc.dram_tensor("wb4", [KT_B[4] * 128, 128], BF16,
                           kind="ExternalInput").ap()
    wbs_d = nc.dram_tensor("wbs", [N_WBS, 128 * 128], BF16,
                           kind="ExternalInput").ap()
    noise_d = nc.dram_tensor("noiseT", [steps * NROWS, BATCH], F8,
                             kind="ExternalInput").ap()
    out_d = nc.dram_tensor("out", [SL[4], BATCH], F32,
                           kind="ExternalOutput").ap()

    t4in_d = nc.dram_tensor("t4in", [SL[4], BATCH], F8, kind="Internal").ap()
    t4g_d = nc.dram_tensor("t4g", [SIZES[4], BATCH], F8,
                           kind="Internal", addr_space="Shared").ap()
    tin_d = nc.dram_tensor("tin", [TROWS, BATCH], F8, kind="Internal").ap()
    tg_d = nc.dram_tensor("tg", [N_CORES * TROWS, BATCH], F8,
                          kind="Internal", addr_space="Shared").ap()
    e32in_d = nc.dram_tensor("e32in", [E32ROWS, BATCH], F8,
                             kind="Internal").ap()
    e32g_d = nc.dram_tensor("e32g", [N_CORES * E32ROWS, BATCH], F8,
                            kind="Internal", addr_space="Shared").ap()
    e10in_d = nc.dram_tensor("e10in", [E10ROWS, BATCH], F8,
                             kind="Internal").ap()
    e10g_d = nc.dram_tensor("e10g", [N_CORES * E10ROWS, BATCH], F8,
                            kind="Internal", addr_space="Shared").ap()
    xin_d = {lo: nc.dram_tensor(f"xin{lo}", [SL[lo], BATCH], BF16,
                                kind="Internal").ap() for lo in range(4)}
    xg_d = {lo: nc.dram_tensor(f"xg{lo}", [SIZES[lo], BATCH], BF16,
                               kind="Internal", addr_space="Shared").ap()
            for lo in range(4)}

    with tile.TileContext(nc) as tc, contextlib.ExitStack() as st:
        apool = st.enter_context(tc.tile_pool(name="acc", bufs=8, space="PSUM"))
        mpool = st.enter_context(tc.tile_pool(name="mv", bufs=4))
        wpool = st.enter_context(tc.tile_pool(name="wbs", bufs=3))
        tpool = st.enter_context(tc.tile_pool(name="tsl", bufs=2))
        npool = st.enter_context(tc.tile_pool(name="nz", bufs=3))
        spool = st.enter_context(tc.tile_pool(name="st", bufs=3))
        sp = st.enter_context(tc.tile_pool(name="state", bufs=1))

        x = {li: sp.tile([128, XT[li] * BATCH], F32, tag=f"x{li}",
                         name=f"x{li}") for li in range(5)}
        e = {lo: sp.tile([128, XT[lo] * BATCH], F8, tag=f"e{lo}",
                         name=f"e{lo}") for lo in range(4)}
        wf = {li: sp.tile([128, KT_F[li] * SLP[li - 1]], BF16, tag=f"wf{li}",
                          name=f"wf{li}") for li in (1, 2, 3, 4)}
        wb4 = sp.tile([128, KT_B[4] * 128], BF16, tag="wb4", name="wb4")

        # ---- resident weights + obs slice (obs pre-scaled x16 on host)
        for li in (1, 2, 3, 4):
            nc.sync.dma_start(
                wf[li][:].rearrange("p (kt f) -> p kt f", f=SLP[li - 1]),
                wf_d[li].rearrange("(kt p) f -> p kt f", p=128))
        nc.sync.dma_start(
            wb4[:].rearrange("p (kt f) -> p kt f", f=128),
            wb4_d.rearrange("(kt p) f -> p kt f", p=128))
        nc.sync.dma_start(
            x[0][:].rearrange("p (t b) -> p t b", b=BATCH),
            obsT_d.rearrange("(t p) b -> p t b", p=128))

        def ag(in_d, out_d):
            nc.gpsimd.collective_compute(
                "AllGather", mybir.AluOpType.bypass, replica_groups=RG,
                ins=[in_d.opt()], outs=[out_d.opt()])

        def mm_accum(banks, lhs_of, src_d, rowof, kt_n, k_real, nt, dt):
            """banks[jt][bc] += lhs_of(kt,kw,jt) @ src_d[rowof(kt)..] chunks."""
            for kt in range(kt_n):
                kw = min(128, k_real - 128 * kt)
                mb = mpool.tile([128, BATCH], dt, tag="mv")
                r0 = rowof(kt)
                nc.sync.dma_start(mb[:kw, :], src_d[r0:r0 + kw])
                for jt in range(nt):
                    lhs = lhs_of(kt, kw, jt)
                    for bc in range(2):
                        nc.tensor.matmul(
                            banks[jt][bc][:, :512], lhs,
                            mb[:kw, bc * 512:bc * 512 + 512],
                            start=(kt == 0), stop=(kt == kt_n - 1))

        def make_banks(nt):
            return [[apool.tile([128, 512], F32, tag="acc", name="acc")
                     for _ in range(2)] for _ in range(nt)]

        def bwd_mm(li, banks, src_d, rowof, dt):
            lo, it_n, kt_n = li - 1, XT[li], KT_B[li]
            if li == 4:
                mm_accum(banks,
                         lambda kt, kw, it: wb4[:kw, kt * 128:kt * 128 + 128],
                         src_d, rowof, kt_n, SIZES[lo], it_n, dt)
                return
            for kt0 in range(0, kt_n, WSEG):
                seg = min(WSEG, kt_n - kt0)
                nblk = seg * it_n
                wt = wpool.tile([128, WSEG * 4 * 128], BF16, tag="wbs")
                nc.sync.dma_start(
                    wt[:].rearrange("p (b f) -> p b f", f=128)[:, :nblk],
                    wbs_d[WB_OFS[li] + kt0 * it_n:
                          WB_OFS[li] + kt0 * it_n + nblk]
                    .rearrange("b (p f) -> p b f", p=128))
                for si in range(seg):
                    kt = kt0 + si
                    mb = mpool.tile([128, BATCH], dt, tag="mv")
                    r0 = rowof(kt)
                    nc.sync.dma_start(mb[:, :], src_d[r0:r0 + 128])
                    for it in range(it_n):
                        lhs = wt[:, (si * it_n + it) * 128:
                                 (si * it_n + it) * 128 + 128]
                        for bc in range(2):
                            nc.tensor.matmul(
                                banks[it][bc][:, :512], lhs,
                                mb[:, bc * 512:bc * 512 + 512],
                                start=(kt == 0), stop=(kt == kt_n - 1))

        def x_update(li, banks):
            """x += GAMMA * (g*dtanh(x) - e)   (all x16-scaled; e_4 == 0)."""
            for it in range(XT[li]):
                for bc in range(2):
                    s0 = it * BATCH + bc * 512
                    t2 = spool.tile([128, 512], F32, tag="t2")
                    nc.scalar.activation(t2[:, :], x[li][:, s0:s0 + 512],
                                         mybir.ActivationFunctionType.Tanh,
                                         scale=1.0 / XS)
                    nc.scalar.activation(t2[:, :], t2[:, :],
                                         mybir.ActivationFunctionType.Square)
                    nc.vector.tensor_scalar(t2[:, :], t2[:, :], -1.0, 1.0,
                                            mybir.AluOpType.mult,
                                            mybir.AluOpType.add)
                    gd = spool.tile([128, 512], F32, tag="gd")
                    nc.vector.tensor_mul(gd[:, :], banks[it][bc][:, :512],
                                         t2[:, :])
                    if li < 4:
                        nc.vector.scalar_tensor_tensor(
                            gd[:, :], e[li][:, s0:s0 + 512], -1.0, gd[:, :],
                            mybir.AluOpType.mult, mybir.AluOpType.add)
                    nc.vector.scalar_tensor_tensor(
                        x[li][:, s0:s0 + 512], gd[:, :], GAMMA,
                        x[li][:, s0:s0 + 512],
                        mybir.AluOpType.mult, mybir.AluOpType.add)

        def t_stage(li):
            """tanh own slice -> fp8 bounce write (gather fired by caller)."""
            for t in range(XT[li]):
                ts = npool.tile([128, BATCH], F8, tag="nz")
                nc.scalar.activation(ts[:, :],
                                     x[li][:, t * BATCH:(t + 1) * BATCH],
                                     mybir.ActivationFunctionType.Tanh,
                                     scale=1.0 / XS)
                if li == 4:
                    nc.sync.dma_start(t4in_d[:SL[4]], ts[:SL[4], :])
                else:
                    r0 = TOFS[li] + t * 128
                    nc.sync.dma_start(tin_d[r0:r0 + 128], ts[:, :])

        def step(i):
            # --- fwd sweeps (li = 4..1), e into fused bounces
            # (consumes t4g/tg produced at the tail of the previous step)
            for li in (4, 3, 2, 1):
                lo = li - 1
                banks = make_banks(XT[lo])
                if li == 4:
                    mm_accum(banks,
                             lambda kt, kw, jt, _li=li, _lo=lo:
                                 wf[_li][:kw, kt * SLP[_lo] + jt * 128:
                                         kt * SLP[_lo] + jt * 128 + 128],
                             t4g_d, lambda kt: 128 * kt, KT_F[4], SIZES[4],
                             XT[lo], F8)
                else:
                    tpc = SL[li] // 128  # t-tiles per core chunk

                    def rowof(kt, _li=li, _t=tpc):
                        return (kt // _t) * TROWS + TOFS[_li] + 128 * (kt % _t)
                    mm_accum(banks,
                             lambda kt, kw, jt, _li=li, _lo=lo:
                                 wf[_li][:kw, kt * SLP[_lo] + jt * 128:
                                         kt * SLP[_lo] + jt * 128 + 128],
                             tg_d, rowof, KT_F[li], SIZES[li], XT[lo], F8)

                # e_lo = (x - pred*XS + noise) as fp8, into fused bounce
                for jt in range(XT[lo]):
                    nz = npool.tile([128, BATCH], F8, tag="nz")
                    r0 = i * NROWS + int(NOFS[lo]) + jt * 128
                    nc.sync.dma_start(nz[:, :], noise_d[r0:r0 + 128])
                    for bc in range(2):
                        s0 = jt * BATCH + bc * 512
                        eb = spool.tile([128, 512], BF16, tag="eb")
                        nc.vector.scalar_tensor_tensor(
                            eb[:, :], banks[jt][bc][:, :512], -XS,
                            x[lo][:, s0:s0 + 512],
                            mybir.AluOpType.mult, mybir.AluOpType.add)
                        nc.vector.scalar_tensor_tensor(
                            e[lo][:, s0:s0 + 512], nz[:, bc * 512:bc * 512 + 512],
                            1.0, eb[:, :],
                            mybir.AluOpType.mult, mybir.AluOpType.add)
                    if lo >= 2:
                        dst, dofs = e32in_d, E32OFS[lo]
                    else:
                        dst, dofs = e10in_d, E10OFS[lo]
                    nc.sync.dma_start(
                        dst[dofs + jt * 128:dofs + (jt + 1) * 128],
                        e[lo][:, jt * BATCH:(jt + 1) * BATCH])
                if li == 3:
                    ag(e32in_d, e32g_d)
                if li == 1:
                    ag(e10in_d, e10g_d)

            # --- bwd sweeps (li = 4..1); right after each x-update, produce
            # next step's tanh slice so the t-gathers overlap the bwd tail
            for li in (4, 3, 2, 1):
                lo = li - 1
                banks = make_banks(XT[li])
                if lo >= 2:
                    src, grows, gofs = e32g_d, E32ROWS, E32OFS[lo]
                else:
                    src, grows, gofs = e10g_d, E10ROWS, E10OFS[lo]
                epc = SL[lo] // 128

                def rowof(kt, _g=grows, _o=gofs, _t=epc):
                    return (kt // _t) * _g + _o + 128 * (kt % _t)
                bwd_mm(li, banks, src, rowof, F8)
                x_update(li, banks)
                if i + 1 < steps:
                    t_stage(li)
                    if li == 4:
                        ag(t4in_d, t4g_d)
                    if li == 1:
                        ag(tin_d, tg_d)

        # ---------------- init: x_li = x_{li-1} @ W_li (bwd-shaped matmuls)
        for li in (1, 2, 3, 4):
            lo = li - 1
            for t in range(XT[lo]):
                xs = mpool.tile([128, BATCH], BF16, tag="mv")
                nc.vector.tensor_copy(xs[:, :],
                                      x[lo][:, t * BATCH:(t + 1) * BATCH])
                rows = min(128, SL[lo] - t * 128)
                nc.sync.dma_start(
                    xin_d[lo][t * 128:t * 128 + rows], xs[:rows, :])
            ag(xin_d[lo], xg_d[lo])
            banks = make_banks(XT[li])
            bwd_mm(li, banks, xg_d[lo], lambda kt: 128 * kt, BF16)
            for it in range(XT[li]):
                for bc in range(2):
                    s0 = it * BATCH + bc * 512
                    nc.scalar.copy(x[li][:, s0:s0 + 512],
                                   banks[it][bc][:, :512])

        # ---------------- settling steps (python-unrolled: collectives
        # inside tc.For_i hardware loops crash this runtime)
        for li in (4, 3, 2, 1):   # prologue: step 0's t-gathers
            t_stage(li)
        ag(t4in_d, t4g_d)
        ag(tin_d, tg_d)
        for i in range(steps):
            step(i)

        nc.sync.dma_start(out_d, x[4][:SL[4], :BATCH])
    nc.finalize()
    return nc


# ---------------------------------------------------------------- host prep

def _host_inputs(obs, Ws, bs, steps):
    import jax, jax.numpy as jnp
    obsf = np.asarray(obs, np.float32)
    cpu = jax.devices("cpu")[0]
    nz_full = {}
    with jax.default_device(cpu):
        nkey = jax.random.key(42)
        for i in range(steps):
            temp = np.float32(1.0 - np.float32(i) / steps)
            for lo in range(4):
                k = jax.random.fold_in(jax.random.fold_in(nkey, i), lo)
                nz = np.asarray(jax.random.normal(k, (BATCH, SIZES[lo]),
                                                  jnp.float32))
                nz_full[(i, lo)] = (nz * np.float32(NOISE_SCALE) * temp
                                    - bs[lo][None, :]) * np.float32(XS)
    in_maps = []
    for c in range(N_CORES):
        m = {}
        m["obsT"] = np.zeros((SLP[0], BATCH), np.float32)
        m["obsT"][:SL[0]] = obsf[:, c * SL[0]:(c + 1) * SL[0]].T * np.float32(XS)
        for li in (1, 2, 3, 4):
            Wt = Ws[li - 1].T  # [SIZES[li], SIZES[li-1]]
            jsl = Wt[:, c * SL[li - 1]:(c + 1) * SL[li - 1]]
            A = np.zeros((KT_F[li] * 128, SLP[li - 1]), np.float32)
            A[:SIZES[li], :SL[li - 1]] = jsl
            m[f"wf{li}"] = A.astype(NPBF16)
        A = np.zeros((KT_B[4] * 128, 128), np.float32)
        A[:SIZES[3], :SL[4]] = Ws[3][:, c * SL[4]:(c + 1) * SL[4]]
        m["wb4"] = A.astype(NPBF16)
        blocks = np.zeros((N_WBS, 128 * 128), np.float32)
        for li in (1, 2, 3):
            isl = Ws[li - 1][:, c * SL[li]:(c + 1) * SL[li]]
            for kt in range(KT_B[li]):
                for it in range(XT[li]):
                    blk = np.zeros((128, 128), np.float32)
                    iw = min(128, SL[li] - it * 128)
                    blk[:, :iw] = isl[kt * 128:kt * 128 + 128,
                                      it * 128:it * 128 + iw]
                    blocks[WB_OFS[li] + kt * XT[li] + it] = blk.reshape(-1)
        m["wbs"] = blocks.astype(NPBF16)
        nzT = np.zeros((steps * NROWS, BATCH), np.float32)
        for i in range(steps):
            for lo in range(4):
                sl = SL[lo]
                nzT[i * NROWS + int(NOFS[lo]):
                    i * NROWS + int(NOFS[lo]) + sl] = \
                    nz_full[(i, lo)][:, c * sl:(c + 1) * sl].T
        m["noiseT"] = nzT.astype(NPF8)
        in_maps.append(m)
    return in_maps


_CACHE = {}


def kernel(**inputs):
    obs = np.asarray(inputs["obs"], np.float32)
    Ws = [np.asarray(inputs[f"W{i}"], np.float32) for i in range(1, 5)]
    bs = [np.asarray(inputs[f"b{i}"], np.float32) for i in range(1, 5)]
    steps = int(inputs["steps"])
    assert obs.shape == (BATCH, SIZES[0])

    if steps not in _CACHE:
        _CACHE[steps] = build(steps)
    nc = _CACHE[steps]

    in_maps = _host_inputs(obs, Ws, bs, steps)
    res = bass_utils.run_bass_kernel_spmd(
        nc, in_maps, core_ids=list(range(N_CORES)), trace=False)
    outT = np.concatenate([res.results[c]["out"] for c in range(N_CORES)], 0)
    return np.ascontiguousarray(outT.T).astype(np.float32) / np.float32(XS)
